# revision 20
# baseline (speedup 1.0000x reference)
"""Trainium2 Bass kernel for AtomPairProjectionBin.

Math: ab = LN(gelu(z @ W_in + b_in)); a, b = split(ab);
x1/x2[b,l,m,k] = sum_{c,d} a[b,l,c] * b[b,m,d] * W[c,d,k] + bias[k]

Strategy:
- Batch-parallel over 8 cores (B=8), no collectives.
- Reassociate the bilinear einsum: R[d,(k,l)] = sum_c W[c,d,k]*a[l,c]
  (stage A, 51 small matmuls), then x[m,(k,l)] = sum_d b[m,d]*R[d,(k,l)]
  (stage B, 13 matmuls of <=512 free).
- LN affine (ln_g/ln_b) and biases b1/b2 are folded on the host into an
  augmented (33,33,51) weight via the ones-row trick, so the device only
  computes the plain LN normalize (two-moment form).
- Stage A/B matmul inputs in bf16 (weights pre-cast on host); PSUM->SBUF
  copies alternate between ACT and DVE; output DMA issued from gpsimd
  (SWDGE) to keep the HWDGE rings free.
"""

import numpy as np

B, L, D_ATOM, DH = 8, 128, 256, 64
H = 32
K1, K2 = 21, 30
KK = K1 + K2  # 51
HA = H + 1  # 33 (augmented with ones row)
OUT_COLS = KK * L  # 6528
LN_EPS = 1e-5
N_GROUPS = 13  # ceil(51/4) groups of 4 k's -> 512-wide chunks

CFG = {
    "a_dt": "bf16",     # stage-A matmul input dtype (wa + at_aug)
    "b_dt": "bf16",     # stage-B matmul input dtype (rt + bt_aug)
    "out_dma": "sync",  # engine issuing output DMAs: sync|gpsimd|alt
    "in_dma": "sync",   # engine issuing input DMAs
    "copyA": "dve",     # engine for stage-A psum copies: dve|act|alt
    "copyB": "act",     # engine for stage-B psum copies: dve|act|alt
    "ln": "moments",    # LN form: v2|moments
    "dma_chunk": 2,     # groups per output DMA (1|2|4)
    "psT_bufs": 2,
    "psA_bufs": 3,
    "psB_bufs": 2,
    "rbufs": 6,
    "xbufs": 4,
    "b_lag": 0,         # groups stage B trails stage A (software pipelining)
    "a_chunk": 1,       # psA banks per A-copy (1: 4 k's, 2: 8 k's)
    "host_zt": True,    # host passes z pre-transposed
    "out_dt": "bf16",    # DRAM output dtype: f32|bf16 (bf16 halves out-DMA bytes)
    "merge_in": True,   # single-DMA z, and w_in+b_in merged into one blob
    "par_in_dma": True, # spread input DMA issue across SP/ACT/DVE
    "ablate": (),       # dev-only: subset of {"prep","A","Acopy","B","Bcopy","dma"}
}

_CACHE = {}


def _patch_tail_drain():
    """Split the Tile tail-drain's sem waits across several Drain instructions.

    The walrus build used on the axon compile path rejects a Drain carrying
    more than one sync wait ("Too many sync wait commands"); the stock
    TileContext._drain_and_barrier attaches one wait per live processor to a
    single Drain.
    """
    import concourse.tile as tile
    from concourse import mybir
    from concourse.vector_clock import ScopedClock

    if getattr(tile.TileContext, "_drain_split_patched", False):
        return

    def _drain_and_barrier(self, tick_clock, wait_clock):
        d0 = self.nc.sync.drain()
        wait_clock.add_sem_waits(d0.ins, ScopedClock({None: tick_clock.global_clock}))
        waits = list(d0.ins.sync_info.on_wait)
        upds = list(d0.ins.sync_info.on_update)
        chunks = [waits[i : i + 1] for i in range(0, len(waits), 1)] or [[]]
        d0.ins.sync_info = mybir.SyncInfo(on_wait=chunks[0], on_update=upds)
        for ch in chunks[1:]:
            dn = self.nc.sync.drain()
            dn.ins.sync_info = mybir.SyncInfo(on_wait=ch, on_update=[])

        self.nc.all_engine_barrier()
        assert self.sems is not None
        popped = self.nc._tile_sem_poison_stack.pop()
        assert popped is self._sem_poison
        self.nc.clear_and_free_semaphores(list(self.sems.allocated().values()))
        self.nc.all_engine_barrier()

    tile.TileContext._drain_and_barrier = _drain_and_barrier
    tile.TileContext._drain_split_patched = True


def _split_multi_waits(nc):
    """Walrus on the axon compile path rejects any instruction carrying more
    than one sync wait. Move excess waits onto same-engine NOPs inserted
    immediately before the offending instruction (safe: the NOP only blocks
    the engine's own stream, which the instruction was about to block on
    anyway)."""
    from concourse import mybir

    n_split = 0
    for fn in nc.m.functions:
        for blk in fn.blocks:
            insts = blk.instructions
            i = 0
            while i < len(insts):
                inst = insts[i]
                si = inst.sync_info
                if si is not None and len(si.on_wait) > 1:
                    waits = list(si.on_wait)
                    inst.sync_info = mybir.SyncInfo(
                        on_wait=waits[-1:], on_update=list(si.on_update)
                    )
                    for j, wt in enumerate(waits[:-1]):
                        nop = mybir.InstNoOp(
                            name=f"{inst.name}-wsplit{j}", ins=[], outs=[]
                        )
                        nop.engine = inst.engine
                        nop.sync_info = mybir.SyncInfo(on_wait=[wt], on_update=[])
                        nc.register_instruction(nop)
                        insts.insert(i, nop)
                        i += 1
                        n_split += 1
                i += 1
    return n_split


def _mdt(name):
    from concourse import mybir

    return {
        "f32": mybir.dt.float32,
        "f32r": mybir.dt.float32r,
        "bf16": mybir.dt.bfloat16,
    }[name]


def _build_nc():
    import concourse.bass as bass
    import concourse.tile as tile
    from concourse import mybir
    from concourse.masks import make_identity

    _patch_tail_drain()

    f32 = mybir.dt.float32
    a_dt = _mdt(CFG["a_dt"])
    b_dt = _mdt(CFG["b_dt"])
    nc = bass.Bass()

    if CFG["host_zt"]:
        z_d = nc.declare_dram_parameter("z", [D_ATOM, L], f32, isOutput=False)
    else:
        z_d = nc.declare_dram_parameter("z", [L, D_ATOM], f32, isOutput=False)
    if CFG["merge_in"]:
        w_in_d = nc.declare_dram_parameter("w_in", [128, 2 * DH + DH], f32, isOutput=False)
        b_in_d = None
    else:
        w_in_d = nc.declare_dram_parameter("w_in", [D_ATOM, DH], f32, isOutput=False)
        b_in_d = nc.declare_dram_parameter("b_in", [1, DH], f32, isOutput=False)
    wa_d = nc.declare_dram_parameter("wa", [HA, KK * HA], a_dt, isOutput=False)
    o_dt = _mdt(CFG["out_dt"]) if CFG["out_dt"] != "f32" else f32
    out_d = nc.declare_dram_parameter("out", [L, OUT_COLS], o_dt, isOutput=True)

    AF = mybir.ActivationFunctionType

    in_eng = {"sync": nc.sync, "gpsimd": nc.gpsimd}[CFG["in_dma"]]

    def out_eng(g):
        if CFG["out_dma"] == "alt":
            return nc.sync if g % 2 == 0 else nc.gpsimd
        return {"sync": nc.sync, "gpsimd": nc.gpsimd}[CFG["out_dma"]]

    with tile.TileContext(nc) as tc:
        with (
            tc.tile_pool(name="const", bufs=1) as constp,
            tc.tile_pool(name="work", bufs=1) as work,
            tc.tile_pool(name="rpool", bufs=CFG["rbufs"]) as rpool,
            tc.tile_pool(name="xout", bufs=CFG["xbufs"]) as xoutp,
            tc.tile_pool(name="psT", bufs=CFG["psT_bufs"], space=bass.MemorySpace.PSUM) as psT,
            tc.tile_pool(name="psA", bufs=CFG["psA_bufs"], space=bass.MemorySpace.PSUM) as psA,
            tc.tile_pool(name="psB", bufs=CFG["psB_bufs"], space=bass.MemorySpace.PSUM) as psB,
        ):
            ident = constp.tile([128, 128], f32)
            make_identity(nc, ident[:])

            eng_z = in_eng
            eng_w = nc.gpsimd if CFG["par_in_dma"] else in_eng
            eng_wa = nc.gpsimd if CFG["par_in_dma"] else in_eng
            if CFG["merge_in"]:
                blob_sb = constp.tile([128, 3 * DH], f32)
                eng_w.dma_start(blob_sb[:], w_in_d[:])
                w_in_sb = blob_sb  # [:, 0:DH]=w0, [:, DH:2DH]=w1, [0, 2DH:3DH]=b_in
            else:
                w_in_sb = constp.tile([128, 2, DH], f32)
                eng_w.dma_start(w_in_sb[:, 0, :], w_in_d[0:128, :])
                eng_w.dma_start(w_in_sb[:, 1, :], w_in_d[128:256, :])
                b_in_sb = constp.tile([1, DH], f32)
                eng_w.dma_start(b_in_sb[:], b_in_d[:])
            wa_sb = constp.tile([HA, KK * HA], a_dt)
            eng_wa.dma_start(wa_sb[:], wa_d[:])

            ones_row = constp.tile([1, 128], f32)
            nc.vector.memset(ones_row[:], 1.0)

            # zT: either host-pretransposed (direct DMA) or via PE transpose
            zt_sb = work.tile([128, 2, 128], f32)
            if CFG["host_zt"] and CFG["merge_in"]:
                zr = z_d.rearrange("(j p) l -> p j l", j=2)
                eng_z.dma_start(zt_sb[:], zr)
            elif CFG["host_zt"]:
                eng_z.dma_start(zt_sb[:, 0, :], z_d[0:128, :])
                eng_z.dma_start(zt_sb[:, 1, :], z_d[128:256, :])
            else:
                z_sb = constp.tile([L, D_ATOM], f32)
                eng_z.dma_start(z_sb[:], z_d[:])
                for j in range(2):
                    pt = psT.tile([128, 128], f32, tag="tp")
                    nc.tensor.transpose(pt[:], z_sb[:, j * 128 : (j + 1) * 128], ident[:])
                    nc.vector.tensor_copy(zt_sb[:, j, :], pt[:])

            # ab_pre = z @ W_in + b_in  (bias via 1-row accumulate matmul)
            ab_ps = psT.tile([L, DH], f32, tag="tp")
            if CFG["merge_in"]:
                w0, w1 = w_in_sb[:, 0:DH], w_in_sb[:, DH : 2 * DH]
                bb_ap = w_in_sb[0:1, 2 * DH : 3 * DH]
            else:
                w0, w1 = w_in_sb[:, 0, :], w_in_sb[:, 1, :]
                bb_ap = b_in_sb[:]
            nc.tensor.matmul(ab_ps[:], zt_sb[:, 0, :], w0, start=True, stop=False)
            nc.tensor.matmul(ab_ps[:], zt_sb[:, 1, :], w1, start=False, stop=False)
            nc.tensor.matmul(ab_ps[:], ones_row[:], bb_ap, start=False, stop=True)

            ab_skip = "prep" in CFG["ablate"]
            # gelu (exact) with row-sum accumulated for the LN mean
            g_sb = work.tile([L, DH], f32)
            gsum = work.tile([L, 1], f32)
            if not ab_skip:
                nc.scalar.activation(g_sb[:], ab_ps[:], AF.Gelu, accum_out=gsum[:])

            eps_t = constp.tile([L, 1], f32)
            nc.vector.memset(eps_t[:], LN_EPS)
            xhat = work.tile([L, DH], f32)
            if ab_skip:
                nc.vector.memset(xhat[:], 0.5)
            elif CFG["ln"] == "moments":
                # var = E[g^2] - mu^2; xhat = (g - mu) * rstd  (fused)
                sq = work.tile([L, DH], f32)
                sqsum = work.tile([L, 1], f32)
                nc.scalar.activation(sq[:], g_sb[:], AF.Square, accum_out=sqsum[:])
                mu = work.tile([L, 1], f32)
                nc.vector.tensor_scalar_mul(mu[:], gsum[:], 1.0 / DH)
                mu2 = work.tile([L, 1], f32)
                nc.vector.tensor_tensor(mu2[:], mu[:], mu[:], op=mybir.AluOpType.mult)
                var = work.tile([L, 1], f32)
                nc.vector.tensor_scalar(
                    var[:], sqsum[:], 1.0 / DH, mu2[:],
                    op0=mybir.AluOpType.mult, op1=mybir.AluOpType.subtract,
                )
                sd = work.tile([L, 1], f32)
                nc.scalar.activation(sd[:], var[:], AF.Sqrt, bias=eps_t[:])
                rstd = work.tile([L, 1], f32)
                nc.vector.reciprocal(rstd[:], sd[:])
                nc.vector.tensor_scalar(
                    xhat[:], g_sb[:], mu[:], rstd[:],
                    op0=mybir.AluOpType.subtract, op1=mybir.AluOpType.mult,
                )
            else:
                mu = work.tile([L, 1], f32)
                nc.vector.tensor_scalar_mul(mu[:], gsum[:], 1.0 / DH)
                xc = work.tile([L, DH], f32)
                nc.vector.tensor_scalar_sub(xc[:], g_sb[:], mu[:])
                sq = work.tile([L, DH], f32)
                vsum = work.tile([L, 1], f32)
                nc.scalar.activation(sq[:], xc[:], AF.Square, accum_out=vsum[:])
                sd = work.tile([L, 1], f32)
                nc.scalar.activation(sd[:], vsum[:], AF.Sqrt, scale=1.0 / DH, bias=eps_t[:])
                rstd = work.tile([L, 1], f32)
                nc.vector.reciprocal(rstd[:], sd[:])
                nc.vector.tensor_scalar_mul(xhat[:], xc[:], rstd[:])

            # transpose the two halves; append ones row (augmented contraction)
            at_aug = work.tile([HA, 128], a_dt)
            bt_aug = work.tile([HA, 128], b_dt)
            for dst in (at_aug, bt_aug):
                off = 0 if dst is at_aug else H
                ptr = psT.tile([H, 128], f32, tag="tp")
                nc.tensor.transpose(ptr[:], xhat[:, off : off + H], ident[:])
                nc.vector.tensor_copy(dst[0:H, :], ptr[:])
                nc.vector.memset(dst[H : H + 1, :], 1.0)

            # stage A + stage B, pipelined per 512-col group.
            # b_lag software-pipelines stage B `b_lag` groups behind stage A so
            # the PE never stalls waiting on a PSUM->SBUF copy.
            def copy_eng(which, g):
                mode = CFG[which]
                if mode == "alt":
                    mode = "dve" if (g % 2 == (0 if which == "copyA" else 1)) else "act"
                return mode

            def do_copy(eng, dst, src):
                if eng == "dve":
                    nc.vector.tensor_copy(dst, src)
                else:
                    nc.scalar.copy(dst, src)

            dchunk = CFG["dma_chunk"]
            achunk = CFG["a_chunk"]
            blag = CFG["b_lag"]
            ab = CFG["ablate"]

            rts = {}
            xo = None
            next_b = 0

            def emit_B(g):
                nonlocal xo
                ks = list(range(g * 4, min(g * 4 + 4, KK)))
                w = len(ks) * 128
                gi = g % dchunk
                if gi == 0:
                    xo = xoutp.tile([128, dchunk * 512], o_dt)
                if "B" not in ab:
                    rt = rts[g // achunk]
                    pb = psB.tile([128, 512], f32)
                    nc.tensor.matmul(
                        pb[:, 0:w], bt_aug[:],
                        rt[:, (g % achunk) * 512 : (g % achunk) * 512 + w],
                        start=True, stop=True,
                    )
                    if "Bcopy" not in ab:
                        do_copy(copy_eng("copyB", g), xo[:, gi * 512 : gi * 512 + w], pb[:, 0:w])
                if "dma" not in ab and (gi == dchunk - 1 or g == N_GROUPS - 1):
                    lo = (g - gi) * 512
                    out_eng(g).dma_start(
                        out_d[:, lo : lo + gi * 512 + w], xo[:, 0 : gi * 512 + w]
                    )

            n_sg = (N_GROUPS + achunk - 1) // achunk
            for sg in range(n_sg):
                gs = list(range(sg * achunk, min((sg + 1) * achunk, N_GROUPS)))
                if "A" not in ab:
                    pa = psA.tile([HA, 4 * achunk, 128], f32)
                    for g in gs:
                        for i, k in enumerate(range(g * 4, min(g * 4 + 4, KK))):
                            nc.tensor.matmul(
                                pa[:, (g - gs[0]) * 4 + i, :],
                                wa_sb[:, k * HA : (k + 1) * HA],
                                at_aug[:],
                                start=True,
                                stop=True,
                            )
                    rt = rpool.tile([HA, achunk * 512], b_dt)
                    rts[sg] = rt
                    if "Acopy" not in ab:
                        nk = sum(len(range(g * 4, min(g * 4 + 4, KK))) for g in gs)
                        do_copy(copy_eng("copyA", sg), rt[:, 0 : nk * 128], pa[:, 0:nk, :])
                while next_b <= gs[-1] - blag:
                    emit_B(next_b)
                    next_b += 1
            while next_b < N_GROUPS:
                emit_B(next_b)
                next_b += 1

    _split_multi_waits(nc)
    nc.finalize()
    return nc


def _host_prep(W_in, b_in, ln_g, ln_b, W1, b1, W2, b2):
    """Fold LN affine + output biases into the augmented (33,33,51) weight."""
    W = np.concatenate(
        [np.asarray(W1, np.float64).reshape(H, H, K1), np.asarray(W2, np.float64).reshape(H, H, K2)],
        axis=2,
    )  # (c, d, k)
    g = np.asarray(ln_g, np.float64)
    bl = np.asarray(ln_b, np.float64)
    ga, gb = g[:H], g[H:]
    ba, bb = bl[:H], bl[H:]
    bias = np.concatenate([np.asarray(b1, np.float64), np.asarray(b2, np.float64)])

    W_aug = np.zeros((HA, HA, KK))
    W_aug[:H, :H, :] = W * ga[:, None, None] * gb[None, :, None]
    W_aug[:H, H, :] = np.einsum("cdk,d->ck", W, bb) * ga[:, None]
    W_aug[H, :H, :] = np.einsum("cdk,c->dk", W, ba) * gb[:, None]
    W_aug[H, H, :] = np.einsum("cdk,c,d->k", W, ba, bb) + bias

    # WA[c', k*33 + d'] = W_aug[c', d', k]
    WA = np.ascontiguousarray(W_aug.transpose(0, 2, 1).reshape(HA, KK * HA)).astype(np.float32)
    if CFG["a_dt"] == "bf16":
        import ml_dtypes

        WA = WA.astype(ml_dtypes.bfloat16)
    if CFG["merge_in"]:
        w = np.asarray(W_in, np.float32)
        blob = np.zeros((128, 3 * DH), np.float32)
        blob[:, 0:DH] = w[0:128]
        blob[:, DH : 2 * DH] = w[128:256]
        blob[0, 2 * DH : 3 * DH] = np.asarray(b_in, np.float32)
        return {"w_in": blob, "wa": WA}
    return {
        "w_in": np.ascontiguousarray(np.asarray(W_in, np.float32)),
        "b_in": np.asarray(b_in, np.float32).reshape(1, DH),
        "wa": WA,
    }


def kernel(z, W_in, b_in, ln_g, ln_b, W1, b1, W2, b2):
    from concourse.bass_utils import run_bass_kernel_spmd

    z = np.ascontiguousarray(np.asarray(z, np.float32))
    shared = _host_prep(W_in, b_in, ln_g, ln_b, W1, b1, W2, b2)

    if "nc" not in _CACHE:
        _CACHE["nc"] = _build_nc()
    nc = _CACHE["nc"]

    zs = [
        np.ascontiguousarray(z[i].T) if CFG["host_zt"] else np.ascontiguousarray(z[i])
        for i in range(B)
    ]
    in_maps = [dict(shared, z=zs[i]) for i in range(B)]
    res = run_bass_kernel_spmd(nc, in_maps, core_ids=list(range(B)))
    _CACHE["last_results"] = res

    outs = np.stack(
        [np.asarray(res.results[i]["out"]).astype(np.float32) for i in range(B)], axis=0
    )
    x = outs.reshape(B, L, KK, L)  # [b, m, k, l]
    x = np.ascontiguousarray(np.transpose(x, (0, 3, 1, 2)))  # [b, l, m, k]
    x1 = np.ascontiguousarray(x[..., :K1])
    x2 = np.ascontiguousarray(x[..., K1:])
    return (x1, x2)
